# revision 27
# baseline (speedup 1.0000x reference)
"""Cross-attention kernel for Trainium2, data-parallel over batch on 8 NeuronCores.

Reference computation (per batch element b):
    lat = LN(latent_q[b]) ; inp = LN(input_kv[b])
    Q = lat @ W_Q ; K = inp @ W_K ; V = inp @ W_V      (8 heads x 128 dims)
    out[b] = softmax(Q K^T / sqrt(128)) V @ W_O

Sharding: batch B=8 -> one batch element per core, zero collectives.

Device program (per core) -- engine-balanced around measured HW rates
(PE matmul+ldweights [128,512] bf16 ~265ns; ACT/DVE/GPSIMD ~0.6-1.2us per
[128,512] vector op). PE is the bottleneck at ~5.3k matmuls:
  - All matmul operands bf16 (fp32 PSUM accumulation). LN gamma/beta folded
    into weights / per-channel bias vectors on the host.
  - x_kv streamed in 32 chunks of 512 rows:
      LN stats (DVE bn_stats) -> rsqrt via ACT Ln+Exp (single activation
      table set for the whole kernel, no ~2.7us table swaps) -> LN apply on
      DVE (one tensor_scalar) -> bf16 x_hat -> DRAM bounce -> DMA-transpose
      to channel-major (keeps transposes off PE/DVE entirely).
  - Emission is software-pipelined: chunk c's K^T/V projection matmuls are
    interleaved head-by-head with chunk c-1's attention (S^T = K Q^T on PE,
    exp on ACT, O' accumulated in PSUM), so PE has projection work to run
    while ACT computes exp.
  - Softmax denominator: exp tiles summed into l_acc on GPSIMD (heads 0-5)
    and DVE (heads 6-7); one ones-matmul partition-reduce per head at the
    end, 1/l broadcast via a rank-1 matmul, normalize, project with W_O.
"""

import numpy as np
import ml_dtypes

import concourse.bass as bass
import concourse.mybir as mybir
import concourse.tile as tile
from concourse import bacc
from concourse.bass_utils import run_bass_kernel_spmd

AF = mybir.ActivationFunctionType
DT = mybir.dt
ALU = mybir.AluOpType

B = 8
LQ = 512
LKV = 16384
DLAT = 1024
DIN = 768
QK_CH = 1024
V_CH = 1024
OUT_CH = 1024
H = 8
DH = 128
P = 128
EPS = 1e-5
SCALE = float(1.0 / np.sqrt(DH))

CHUNK = 512               # kv rows per chunk
N_KV_T = CHUNK // P       # 4
N_LQ_T = LQ // P          # 4
N_LAT_S = DLAT // P       # 8
N_IN_S = DIN // P         # 6
N_VC_S = V_CH // P        # 8


def build_program(lkv=LKV, reps=1):
    """Build the per-core Bass program. reps>1 wraps the body in a HW loop
    (each iteration recomputes the full output; used for wall-clock timing)."""
    n_chunks = lkv // CHUNK

    nc = bacc.Bacc()
    lq_d = nc.dram_tensor("lq", [LQ, DLAT], DT.float32, kind="ExternalInput")
    xkv_d = nc.dram_tensor("xkv", [lkv, DIN], DT.float32, kind="ExternalInput")
    wq_d = nc.dram_tensor("wq", [DLAT, QK_CH], DT.bfloat16, kind="ExternalInput")
    wk_d = nc.dram_tensor("wk", [DIN, QK_CH], DT.bfloat16, kind="ExternalInput")
    wv_d = nc.dram_tensor("wv", [DIN, V_CH], DT.bfloat16, kind="ExternalInput")
    wo_d = nc.dram_tensor("wo", [V_CH, OUT_CH], DT.bfloat16, kind="ExternalInput")
    tq_d = nc.dram_tensor("tq", [P, H], DT.float32, kind="ExternalInput")
    tk_d = nc.dram_tensor("tk", [P, H], DT.float32, kind="ExternalInput")
    tvb_d = nc.dram_tensor("tvb", [P, V_CH], DT.bfloat16, kind="ExternalInput")
    out_d = nc.dram_tensor("out", [LQ, OUT_CH], DT.float32, kind="ExternalOutput")

    with tile.TileContext(nc) as tc:
        with (
            tc.tile_pool(name="weights", bufs=1) as wpool,
            tc.tile_pool(name="persist", bufs=1) as perpool,
            tc.tile_pool(name="xin", bufs=2) as xpool,
            tc.tile_pool(name="xn", bufs=2) as xnpool,
            tc.tile_pool(name="xnt", bufs=2) as xntpool,
            tc.tile_pool(name="kt", bufs=2) as ktpool,
            tc.tile_pool(name="vt", bufs=2) as vpool,
            tc.tile_pool(name="pt", bufs=4) as ptpool,
            tc.tile_pool(name="stats", bufs=3) as stats_pool,
            tc.tile_pool(name="dram", bufs=2, space="DRAM") as dram_pool,
            tc.tile_pool(name="kvpsum", bufs=2, space="PSUM") as kvpsum,
            tc.tile_pool(name="spsum", bufs=4, space="PSUM") as spsum,
            tc.tile_pool(name="opsum", bufs=2, space="PSUM") as opsum,
        ):
            # ---- weight/constant tiles (DMAs emitted inside body) ----
            wq_sb = wpool.tile([P, N_LAT_S, QK_CH], DT.bfloat16)
            wk_sb = wpool.tile([P, N_IN_S, QK_CH], DT.bfloat16)
            wv_sb = wpool.tile([P, N_IN_S, V_CH], DT.bfloat16)
            wo_sb = wpool.tile([P, N_VC_S, OUT_CH], DT.bfloat16)
            tq_sb = wpool.tile([P, H], DT.float32)
            tk_sb = wpool.tile([P, H], DT.float32)
            tvb_sb = wpool.tile([P, V_CH], DT.bfloat16)
            ones_f32 = wpool.tile([P, 1], DT.float32)
            ones_row = wpool.tile([1, P], DT.float32)
            eps_sb = wpool.tile([P, 1], DT.float32)

            q_sb = perpool.tile([P, H, LQ], DT.bfloat16)
            o_acc = perpool.tile([P, H, LQ], DT.float32)
            l_acc = perpool.tile([P, H, LQ], DT.float32)

            def ln_stats(x_ap, n_sub, width):
                """LN stats for [128, n_sub, width] fp32 -> (inv, nmi) [128, n_sub]."""
                half = width // 2
                st = stats_pool.tile([P, n_sub, 12], DT.float32, tag="bnst")
                mv = stats_pool.tile([P, n_sub, 2], DT.float32, tag="bnmv")
                for t in range(n_sub):
                    nc.vector.bn_stats(st[:, t, 0:6], x_ap[:, t, 0:half])
                    nc.vector.bn_stats(st[:, t, 6:12], x_ap[:, t, half:width])
                    nc.vector.bn_aggr(mv[:, t, :], st[:, t, :])
                lnv = stats_pool.tile([P, n_sub], DT.float32, tag="bnln")
                nc.scalar.activation(lnv[:], mv[:, :, 1], AF.Ln, bias=eps_sb[:])
                inv = stats_pool.tile([P, n_sub], DT.float32, tag="bninv")
                nc.scalar.activation(inv[:], lnv[:], AF.Exp, scale=-0.5)
                nmi = stats_pool.tile([P, n_sub], DT.float32, tag="bnnmi")
                nc.vector.tensor_mul(nmi[:], mv[:, :, 0], inv[:])
                nc.vector.tensor_scalar_mul(nmi[:], nmi[:], -1.0)
                return inv, nmi

            def body():
                # weights spread over the three DMA queues (SWDGE + both
                # HWDGE issuers) so they load in parallel with the first
                # chunk's data on the sync queue
                nc.gpsimd.dma_start(wk_sb[:], wk_d[:].rearrange("(s p) n -> p s n", p=P))
                nc.gpsimd.dma_start(wq_sb[:], wq_d[:].rearrange("(s p) n -> p s n", p=P))
                nc.scalar.dma_start(wv_sb[:], wv_d[:].rearrange("(s p) n -> p s n", p=P))
                nc.scalar.dma_start(wo_sb[:], wo_d[:].rearrange("(s p) n -> p s n", p=P))
                nc.scalar.dma_start(tq_sb[:], tq_d[:])
                nc.gpsimd.dma_start(tk_sb[:], tk_d[:])
                nc.scalar.dma_start(tvb_sb[:], tvb_d[:])
                nc.gpsimd.memset(ones_f32[:], 1.0)
                nc.gpsimd.memset(ones_row[:], 1.0)
                nc.gpsimd.memset(eps_sb[:], EPS)
                nc.gpsimd.memset(o_acc[:], 0.0)
                nc.gpsimd.memset(l_acc[:], 0.0)

                # ---------- main loop over kv chunks ----------
                # Software-pipelined emission: the load/LN/transpose stage for
                # chunk c+1 is emitted before chunk c's projection+attention
                # block, and chunk c's K/V projections are interleaved
                # head-by-head with chunk c-1's attention, so PE always has
                # projection matmuls to run while ACT does exp and the LN/DMA
                # chain hides behind compute.
                xkv_r = xkv_d[:].rearrange("(c t p) ch -> c p t ch", t=N_KV_T, p=P)

                def stage_load(c):
                    x_t = xpool.tile([P, N_KV_T, DIN], DT.float32, tag="x")
                    nc.sync.dma_start(x_t[:], xkv_r[c])
                    inv, nmi = ln_stats(x_t, N_KV_T, DIN)
                    xn_t = xnpool.tile([P, N_KV_T, DIN], DT.bfloat16, tag="xn")
                    for t in range(N_KV_T):
                        nc.vector.tensor_scalar(
                            xn_t[:, t, :], x_t[:, t, :],
                            inv[:, t : t + 1], nmi[:, t : t + 1], ALU.mult, ALU.add,
                        )
                    xnd = dram_pool.tile([CHUNK, DIN], DT.bfloat16, tag="xnd")
                    nc.sync.dma_start(
                        xnd[:].rearrange("(t p) ch -> p t ch", p=P), xn_t[:]
                    )
                    xnT = xntpool.tile([P, N_IN_S, CHUNK], DT.bfloat16)
                    nc.sync.dma_start_transpose(xnT[:], xnd[:])
                    return xnT

                xnT_cur = stage_load(0)

                # ---------- prologue: latent LN -> DMA transpose -> Q^T ----------
                latd = dram_pool.tile([LQ, DLAT], DT.bfloat16, tag="latd")
                lq_r = lq_d[:].rearrange("(t p) n -> t p n", p=P)
                latd_r = latd[:].rearrange("(t p) n -> t p n", p=P)
                for t in range(N_LQ_T):
                    lat_t = xpool.tile([P, 1, DLAT], DT.float32, tag="x")
                    nc.sync.dma_start(lat_t[:, 0, :], lq_r[t])
                    inv, nmi = ln_stats(lat_t, 1, DLAT)
                    latn = xnpool.tile([P, DLAT], DT.bfloat16, tag="xn")
                    nc.vector.tensor_scalar(
                        latn[:], lat_t[:, 0, :],
                        inv[:, 0:1], nmi[:, 0:1], ALU.mult, ALU.add,
                    )
                    nc.sync.dma_start(latd_r[t], latn[:])
                latnT = ktpool.tile([P, N_LAT_S, LQ], DT.bfloat16, tag="kT")
                nc.sync.dma_start_transpose(latnT[:], latd[:])
                for h in range(H):
                    qps = kvpsum.tile([P, LQ], DT.float32, tag="kv")
                    for s in range(N_LAT_S):
                        nc.tensor.matmul(
                            qps[:],
                            wq_sb[:, s, h * DH : (h + 1) * DH],
                            latnT[:, s, :],
                            start=(s == 0),
                            stop=(s == N_LAT_S - 1),
                        )
                    nc.vector.tensor_scalar(
                        q_sb[:, h, :], qps[:],
                        tq_sb[:, h : h + 1], None, ALU.add,
                    )
                def attn_head(kT, v_t, h):
                    ops = opsum.tile([P, LQ], DT.float32, tag="o")
                    l_eng = nc.gpsimd if h < 6 else nc.vector
                    for t in range(N_KV_T):
                        sps = spsum.tile([P, LQ], DT.float32, tag="s")
                        nc.tensor.matmul(
                            sps[:],
                            kT[:, h, t * P : (t + 1) * P],
                            q_sb[:, h, :],
                            start=True,
                            stop=True,
                        )
                        pT = ptpool.tile([P, LQ], DT.bfloat16)
                        nc.scalar.activation(pT[:], sps[:], AF.Exp, scale=SCALE)
                        l_eng.tensor_add(l_acc[:, h, :], l_acc[:, h, :], pT[:])
                        nc.tensor.matmul(
                            ops[:],
                            v_t[:, t, h * DH : (h + 1) * DH],
                            pT[:],
                            start=(t == 0),
                            stop=(t == N_KV_T - 1),
                        )
                    nc.vector.tensor_add(o_acc[:, h, :], o_acc[:, h, :], ops[:])

                prev = None
                for c in range(n_chunks):
                    xnT = xnT_cur
                    if c + 1 < n_chunks:
                        xnT_cur = stage_load(c + 1)
                    kT = ktpool.tile([P, H, CHUNK], DT.bfloat16, tag="kT")
                    v_t = vpool.tile([P, N_KV_T, V_CH], DT.bfloat16)
                    for h in range(H):
                        # K^T head h of chunk c
                        kps = kvpsum.tile([P, CHUNK], DT.float32, tag="kv")
                        for s in range(N_IN_S):
                            nc.tensor.matmul(
                                kps[:],
                                wk_sb[:, s, h * DH : (h + 1) * DH],
                                xnT[:, s, :],
                                start=(s == 0),
                                stop=(s == N_IN_S - 1),
                            )
                        nc.vector.tensor_scalar(
                            kT[:, h, :], kps[:],
                            tk_sb[:, h : h + 1], None, ALU.add,
                        )
                        # V slice (t, half) = (h//2, h%2) of chunk c
                        t, nf = h // 2, h % 2
                        vps = kvpsum.tile([P, 512], DT.float32, tag="kv")
                        for s in range(N_IN_S):
                            nc.tensor.matmul(
                                vps[:],
                                xnT[:, s, t * P : (t + 1) * P],
                                wv_sb[:, s, nf * 512 : (nf + 1) * 512],
                                start=(s == 0),
                                stop=(s == N_IN_S - 1),
                            )
                        nc.vector.tensor_add(
                            v_t[:, t, nf * 512 : (nf + 1) * 512],
                            vps[:],
                            tvb_sb[:, nf * 512 : (nf + 1) * 512],
                        )
                        # attention head h of chunk c-1
                        if prev is not None:
                            attn_head(prev[0], prev[1], h)
                    prev = (kT, v_t)
                for h in range(H):
                    attn_head(prev[0], prev[1], h)

                # ---------- epilogue: normalize, project with W_O ----------
                o_n = ktpool.tile([P, H, LQ], DT.bfloat16, tag="kT")
                for h in range(H):
                    lred = spsum.tile([1, LQ], DT.float32, tag="s")
                    nc.tensor.matmul(
                        lred[:], ones_f32[:], l_acc[:, h, :], start=True, stop=True
                    )
                    # reuse row 0 of the (fully consumed) accumulator as 1/l
                    nc.vector.reciprocal(l_acc[0:1, h, :], lred[:])
                for h in range(H):
                    bps = opsum.tile([P, LQ], DT.float32, tag="o")
                    nc.tensor.matmul(
                        bps[:], ones_row[:], l_acc[0:1, h, :], start=True, stop=True
                    )
                    nc.vector.tensor_mul(o_n[:, h, :], o_acc[:, h, :], bps[:])
                for nf in range(OUT_CH // 512):
                    out_sb = xnpool.tile([P, N_LQ_T, 512], DT.float32, tag="xn")
                    for qt in range(N_LQ_T):
                        octile = kvpsum.tile([P, 512], DT.float32, tag="kv")
                        for s in range(N_VC_S):
                            nc.tensor.matmul(
                                octile[:],
                                o_n[:, s, qt * P : (qt + 1) * P],
                                wo_sb[:, s, nf * 512 : (nf + 1) * 512],
                                start=(s == 0),
                                stop=(s == N_VC_S - 1),
                            )
                        nc.vector.tensor_copy(out_sb[:, qt, :], octile[:])
                    nc.sync.dma_start(
                        out_d[:].rearrange("(t p) n -> p t n", p=P)[
                            :, :, nf * 512 : (nf + 1) * 512
                        ],
                        out_sb[:],
                    )

            if reps == 1:
                body()
            else:
                with tc.For_i(0, reps, 1) as _i:
                    body()

    nc.compile()
    return nc


def host_prep(W_Q, W_K, W_V, W_O, ln_lat_g, ln_lat_b, ln_in_g, ln_in_b):
    """Fold LN affine params into weights; returns device input dict pieces."""
    bf16 = ml_dtypes.bfloat16
    wq = (ln_lat_g[:, None].astype(np.float64) * W_Q.astype(np.float64)).astype(bf16)
    wk = (ln_in_g[:, None].astype(np.float64) * W_K.astype(np.float64)).astype(bf16)
    wv = (ln_in_g[:, None].astype(np.float64) * W_V.astype(np.float64)).astype(bf16)
    wo = W_O.astype(bf16)
    tq = (ln_lat_b.astype(np.float64) @ W_Q.astype(np.float64)).astype(np.float32)
    tk = (ln_in_b.astype(np.float64) @ W_K.astype(np.float64)).astype(np.float32)
    tv = (ln_in_b.astype(np.float64) @ W_V.astype(np.float64)).astype(np.float32)
    # t_q/t_k laid out [dh-partition, head]; t_v broadcast to all partitions
    tq_l = np.ascontiguousarray(tq.reshape(H, DH).T)
    tk_l = np.ascontiguousarray(tk.reshape(H, DH).T)
    tvb = np.ascontiguousarray(np.broadcast_to(tv.astype(bf16), (P, V_CH)))
    return dict(wq=wq, wk=wk, wv=wv, wo=wo, tq=tq_l, tk=tk_l, tvb=tvb)


_prog_cache = {}


def _get_program():
    key = "main"
    if key not in _prog_cache:
        _prog_cache[key] = build_program()
    return _prog_cache[key]


def kernel(latent_q, input_kv, W_Q, W_K, W_V, W_O,
           ln_lat_g, ln_lat_b, ln_in_g, ln_in_b):
    shared = host_prep(W_Q, W_K, W_V, W_O, ln_lat_g, ln_lat_b, ln_in_g, ln_in_b)
    nc = _get_program()
    in_maps = [
        dict(
            lq=np.ascontiguousarray(latent_q[b]),
            xkv=np.ascontiguousarray(input_kv[b]),
            **shared,
        )
        for b in range(B)
    ]
    res = run_bass_kernel_spmd(nc, in_maps, list(range(B)))
    out = np.stack([res.results[b]["out"] for b in range(B)])
    return out.astype(np.float32)
